# revision 3
# baseline (speedup 1.0000x reference)
"""Distributed self-attention kernel for one TRN2 chip (8 NeuronCores), v3.

Problem: b=2, n=2048, d=1024, 16 heads x 64 dim, fp32 in/out.

Sharding: core i -> batch i//4, head group i%4 (2 pairs of heads).
Host sums the 4 partial (n, d) outputs per batch while unsharding.

Key layout choices:
  - qT/kT stored per-head with the 64 head dims DUPLICATED on both
    partition halves (written by DVE partition-shifted copies straight from
    the projection psum). Sim matmuls then alternate PE row-tiles T0/T8 by
    jt parity: two concurrent 64-row matmuls + overlapped LDWEIGHTS.
  - V tile carries 64 replicated ones-columns (M=128): the av matmul
    broadcasts the softmax denominators into psum rows 64-127 for free,
    so normalization is just DVE reciprocal[64,1024] + mul -- no gpsimd
    broadcast, no single-lane [1,N] ops.
  - One [128,1024] ACT exp per jt (ACT is the roofline engine); av
    accumulates [128,1024] psum across all 16 jt.
  - wo accumulates both pairs in one psum chain; out DMA'd incrementally
    (first n-half mid-attention, second at the tail).
  - PSUM budget: sim 2x2 + av 2 + mm 2 = 8 banks.
"""

import sys

if "/opt/trn_rl_repo" not in sys.path:
    sys.path.append("/opt/trn_rl_repo")

import ml_dtypes
import numpy as np

import concourse.bass as bass
import concourse.tile as tile
from concourse.tile import add_dep_helper
from concourse import bacc, mybir
from concourse.bass_utils import run_bass_kernel_spmd

F32 = mybir.dt.float32
BF16 = mybir.dt.bfloat16
AF = mybir.ActivationFunctionType
NPBF16 = ml_dtypes.bfloat16

P = 128
B = 2
N = 2048
D = 1024
H = 16
HD = 64
NCORES = 8
G = 4            # cores per batch
HPC = H // G     # heads per core = 4
C = HPC * HD     # per-core inner slice = 256
IC = 512         # psum free-dim chunk (one bank)
C2 = 1024        # attention query-chunk width (ACT window)
NIC = N // IC    # 4
JT = N // P      # 16 key tiles
DK = D // P      # 8 contraction chunks

DUP_SIM = True   # duplicated qT/kT halves -> T0/T8 alternating sims

_compiled = {}


def _emit(tc):
    nc = tc.nc
    tokT_e = nc.dram_tensor("tokT", [D, N], BF16, kind="ExternalInput")
    wq_e = nc.dram_tensor("wq", [D, C], BF16, kind="ExternalInput")
    wk_e = nc.dram_tensor("wk", [D, C], BF16, kind="ExternalInput")
    wv_e = nc.dram_tensor("wv", [D, C], BF16, kind="ExternalInput")
    wo_e = nc.dram_tensor("wo", [C, D], BF16, kind="ExternalInput")
    out_e = nc.dram_tensor("out", [N, D], BF16, kind="ExternalOutput")

    from contextlib import ExitStack

    with ExitStack() as ctx:
        # PSUM: sim 2x2 banks + av 2 banks + mm 2 banks = 8 banks
        ps_sim = ctx.enter_context(tc.tile_pool(name="ps_sim", bufs=2, space="PSUM"))
        ps_av = ctx.enter_context(tc.tile_pool(name="ps_av", bufs=1, space="PSUM"))
        ps_mm = ctx.enter_context(tc.tile_pool(name="ps_mm", bufs=2, space="PSUM"))

        tokp = ctx.enter_context(tc.tile_pool(name="tok", bufs=1))
        wp = ctx.enter_context(tc.tile_pool(name="w", bufs=1))
        qkh = ctx.enter_context(tc.tile_pool(name="qkh", bufs=1))
        vp = ctx.enter_context(tc.tile_pool(name="v", bufs=1))
        etp = ctx.enter_context(tc.tile_pool(name="et", bufs=6))
        attp = ctx.enter_context(tc.tile_pool(name="attnT", bufs=1))
        outp = ctx.enter_context(tc.tile_pool(name="osb", bufs=4))
        small = ctx.enter_context(tc.tile_pool(name="small", bufs=2))

        # DMA pushes: sync + gpsimd steady-state; scalar also in the
        # prologue (before the first exp) where ACT is idle anyway.
        dma_engs = [nc.sync, nc.gpsimd]
        pro_engs = [nc.sync, nc.gpsimd, nc.scalar]
        dma_i = [0]

        def dma(out, in_, prologue=False):
            engs = pro_engs if prologue else dma_engs
            eng = engs[dma_i[0] % len(engs)]
            dma_i[0] += 1
            eng.dma_start(out=out, in_=in_)

        # ---------------- input DMAs (order = arrival priority) ----------
        tok = [tokp.tile([P, N], BF16, tag=f"tok{dk}", name=f"tok{dk}")
               for dk in range(DK)]
        wq_sb = [wp.tile([P, C], BF16, tag=f"wq{dk}", name=f"wqs{dk}")
                 for dk in range(DK)]
        wk_sb = [wp.tile([P, C], BF16, tag=f"wk{dk}", name=f"wks{dk}")
                 for dk in range(DK)]
        wv_sb = [wp.tile([P, C], BF16, tag=f"wv{dk}", name=f"wvs{dk}")
                 for dk in range(DK)]
        wo_sb = [wp.tile([P, D], BF16, tag=f"wo{kk}", name=f"wos{kk}")
                 for kk in range(2)]

        qtr = N // 2
        for ic in range(2):  # first-needed-first: ic0 tok, wk, wq, ic1...
            for dk in range(DK):
                dma(tok[dk][:, IC * ic:IC * (ic + 1)],
                    tokT_e[P * dk:P * (dk + 1), IC * ic:IC * (ic + 1)],
                    prologue=True)
            for dk in range(DK):
                dma((wk_sb if ic == 0 else wq_sb)[dk][:],
                    (wk_e if ic == 0 else wq_e)[P * dk:P * (dk + 1), :],
                    prologue=True)
        for dk in range(DK):
            dma(wv_sb[dk][:], wv_e[P * dk:P * (dk + 1), :], prologue=True)
        for ic in range(2, 4):
            for dk in range(DK):
                dma(tok[dk][:, IC * ic:IC * (ic + 1)],
                    tokT_e[P * dk:P * (dk + 1), IC * ic:IC * (ic + 1)],
                    prologue=True)
        for dk in range(DK):
            dma(tok[dk][:, qtr:N], tokT_e[P * dk:P * (dk + 1), qtr:N],
                prologue=True)
        for kk in range(2):
            dma(wo_sb[kk][:], wo_e[P * kk:P * (kk + 1), :], prologue=True)

        # HAM warmup: ~160 tiny junk matmuls spanning the input-DMA wait so
        # the PE clock is already 2.4GHz when the first projection chain
        # lands (cold eager phase previously ran at 1.2GHz). Results unused;
        # sim-slot ring cycles freely (no readers).
        warm_t = small.tile([P, HD], BF16, tag="warm", name="warm_t")
        nc.vector.memset(warm_t[:], 0.25)
        for _ in range(160):
            wps = ps_sim.tile([HD, HD], F32, tag="sim", name="wps")
            nc.tensor.matmul(wps[:], lhsT=warm_t[:, 0:HD],
                             rhs=warm_t[:, 0:HD], start=True, stop=True)

        # ---------------- background (deferred PE) machinery -------------
        # bg entries are (key, thunk); key (non-None on a chain's last
        # thunk) records that chain as emitted. need(key) force-drains bg
        # until the producer chain is emitted, so consumers can never be
        # emitted before their producers exist (the tile framework only
        # orders instructions that have been emitted).
        bg = []
        last_mm = [None]
        emitted = set()

        def drain_bg(n):
            for _ in range(n):
                if bg:
                    key, t = bg.pop(0)
                    t()
                    if key is not None:
                        emitted.add(key)

        def need(key):
            while key not in emitted:
                assert bg, f"need({key}) but bg empty"
                drain_bg(1)

        # per-head qT/kT with head dims duplicated on both partition halves
        qTh = [qkh.tile([P, N], BF16, tag=f"qTh{h}", name=f"qTh{h}")
               for h in range(HPC)]
        kTh = [qkh.tile([P, N], BF16, tag=f"kTh{h}", name=f"kTh{h}")
               for h in range(HPC)]

        def qk_chain(w_sb, head_dst, p, ic, eager):
            """One 8-dk projection chain -> 4 partition-half DVE copies."""
            state = {}

            def mk(dk):
                def thunk():
                    if dk == 0:
                        state["ps"] = ps_mm.tile(
                            [P, IC], F32, tag="mm", name="ps")
                    mm = nc.tensor.matmul(
                        state["ps"][:],
                        lhsT=w_sb[dk][:, P * p:P * (p + 1)],
                        rhs=tok[dk][:, IC * ic:IC * (ic + 1)],
                        start=(dk == 0),
                        stop=(dk == DK - 1),
                    )
                    if last_mm[0] is not None and not eager:
                        add_dep_helper(mm.ins, last_mm[0].ins, sync=False,
                                       reason="bg order")
                    if dk == DK - 1:
                        sl = slice(IC * ic, IC * (ic + 1))
                        ps = state["ps"]
                        for hh in range(2):
                            h = 2 * p + hh
                            src = ps[HD * hh:HD * (hh + 1), :]
                            nc.vector.tensor_copy(head_dst[h][0:HD, sl], src)
                            nc.vector.tensor_copy(head_dst[h][HD:P, sl], src)
                return thunk

            key = ("q" if w_sb is wq_sb else "k", p, ic)
            for dk in range(DK):
                t = mk(dk)
                if eager:
                    t()
                    if dk == DK - 1:
                        emitted.add(key)
                else:
                    bg.append((key if dk == DK - 1 else None, t))

        # V for all 4 heads: [jt, head, 128] -- cols 64:128 are ones so the
        # av matmul replicates the softmax sums across psum rows 64-127.
        vtile = vp.tile([P, JT, HPC, 2 * HD], BF16, tag="v", name="vtile")
        nc.vector.memset(vtile[:, :, :, HD:2 * HD], 1.0)

        def emit_v(jt, eager):
            state = {}

            def mk_v(dk):
                def thunk():
                    if dk == 0:
                        state["ps"] = ps_mm.tile(
                            [P, HPC, HD], F32, tag="mm", name="ps")
                    mm = nc.tensor.matmul(
                        state["ps"][:],
                        lhsT=tok[dk][:, P * jt:P * (jt + 1)],
                        rhs=wv_sb[dk][:],
                        start=(dk == 0),
                        stop=(dk == DK - 1),
                    )
                    if last_mm[0] is not None and not eager:
                        add_dep_helper(mm.ins, last_mm[0].ins, sync=False,
                                       reason="bg order")
                    if dk == DK - 1:
                        nc.vector.tensor_copy(
                            vtile[:, jt, :, 0:HD], state["ps"][:])
                return thunk

            for dk in range(DK):
                t = mk_v(dk)
                if eager:
                    t()
                    if dk == DK - 1:
                        emitted.add(("v", jt))
                else:
                    bg.append((("v", jt) if dk == DK - 1 else None, t))

        attnT = [attp.tile([P, N], BF16, tag=f"attnT{i}", name=f"attnT{i}")
                 for i in range(2)]

        # ---------------- attention pipeline ----------------
        # Global FIFO of deferred av matmuls: a block's last avs and its
        # normalize run during the NEXT block's first iterations, so the
        # PE/ACT pipeline never drains at block boundaries. Entries are
        # (av_closure, norm_closure_or_None); the normalize fires right
        # after its block's jt==15 av.
        pend = []

        def pop_pend(n):
            for _ in range(n):
                if pend:
                    fn, norm = pend.pop(0)
                    fn()
                    if norm is not None:
                        norm()

        def attn_block(p, hh, c2, drain):
            h = 2 * p + hh
            avp = ps_av.tile([P, C2], F32, tag="av", name="avp")

            def sim_act(jt):
                need(("k", p, jt // 4))
                need(("q", p, 2 * c2))
                need(("q", p, 2 * c2 + 1))
                reg = ps_sim.tile([P, C2], F32, tag="sim", name="reg")
                r0 = HD * (jt % 2) if DUP_SIM else 0
                for s in range(2):
                    mm = nc.tensor.matmul(
                        reg[:, IC * s:IC * (s + 1)],
                        lhsT=kTh[h][r0:r0 + HD, P * jt:P * (jt + 1)],
                        rhs=qTh[h][r0:r0 + HD,
                                   C2 * c2 + IC * s:C2 * c2 + IC * (s + 1)],
                        start=True,
                        stop=True,
                    )
                    last_mm[0] = mm
                et = etp.tile([P, C2], BF16, tag="et", name="et")
                nc.scalar.activation(et[:], reg[:], AF.Exp)
                return et

            def mk_av(jt, et):
                def fn():
                    need(("v", jt))
                    for s in range(2):
                        mm = nc.tensor.matmul(
                            avp[:, IC * s:IC * (s + 1)],
                            lhsT=vtile[:, jt, h, :],
                            rhs=et[:, IC * s:IC * (s + 1)],
                            start=(jt == 0),
                            stop=(jt == JT - 1),
                        )
                        last_mm[0] = mm
                return fn

            def mk_norm():
                def fn():
                    # psum rows 64-127 hold the denominators replicated.
                    # reciprocal_approx_fast mis-computes on partition-shifted
                    # inputs, so stage the sums at base partition 0 first.
                    sums = small.tile([HD, C2], F32, tag="sums", name="sums")
                    nc.vector.tensor_copy(sums[:], avp[HD:P, :])
                    rec64 = small.tile([HD, C2], F32, tag="rec64",
                                       name="rec64")
                    nc.vector.reciprocal_approx_fast(out=rec64[:], in_=sums[:])
                    nc.vector.tensor_mul(
                        attnT[p][HD * hh:HD * (hh + 1),
                                 C2 * c2:C2 * (c2 + 1)],
                        avp[0:HD, :],
                        rec64[:],
                    )
                return fn

            for i in range(JT // 2):
                et0 = sim_act(2 * i)
                et1 = sim_act(2 * i + 1)
                pend.append((mk_av(2 * i, et0), None))
                pend.append((mk_av(2 * i + 1, et1),
                             mk_norm() if 2 * i + 1 == JT - 1 else None))
                pop_pend(len(pend) - 4)
                drain_bg(drain)

        # ---------------- wo chains (deferred, A/B split) ----------------
        # A: pair0 contribution -> SBUF staging (can run as soon as attnT[0]
        # has the columns). B: pair1 contribution + add + out DMA.
        halfA = {}

        def queue_wo_A(half):
            for j in range(8):
                nt = 8 * half + j
                for do in range(2):
                    def mk_a(nt, do):
                        def thunk():
                            ps = ps_mm.tile([P, IC], F32, tag="mm", name="ps")
                            mm = nc.tensor.matmul(
                                ps[:],
                                lhsT=attnT[0][:, P * nt:P * (nt + 1)],
                                rhs=wo_sb[0][:, IC * do:IC * (do + 1)],
                                start=True,
                                stop=True,
                            )
                            if last_mm[0] is not None:
                                add_dep_helper(mm.ins, last_mm[0].ins,
                                               sync=False, reason="wo order")
                            ha = outp.tile([P, IC], BF16, tag="halfA",
                                           name="ha", bufs=32)
                            nc.vector.tensor_copy(ha[:], ps[:])
                            halfA[(nt, do)] = ha
                        return thunk

                    bg.append((None, mk_a(nt, do)))

        def queue_wo_B(half):
            for j in range(8):
                nt = 8 * half + j
                for do in range(2):
                    def mk_b(nt, do):
                        def thunk():
                            ps = ps_mm.tile([P, IC], F32, tag="mm", name="ps")
                            mm = nc.tensor.matmul(
                                ps[:],
                                lhsT=attnT[1][:, P * nt:P * (nt + 1)],
                                rhs=wo_sb[1][:, IC * do:IC * (do + 1)],
                                start=True,
                                stop=True,
                            )
                            if last_mm[0] is not None:
                                add_dep_helper(mm.ins, last_mm[0].ins,
                                               sync=False, reason="wo order")
                            osb = outp.tile([P, IC], BF16, tag="osb",
                                            name="osb")
                            nc.vector.tensor_add(osb[:], ps[:],
                                                 halfA[(nt, do)][:])
                            dma(out_e[P * nt:P * (nt + 1),
                                      IC * do:IC * (do + 1)], osb[:])
                        return thunk

                    bg.append((None, mk_b(nt, do)))

        # ---------------- emission schedule ----------------
        # eager: the minimum to start block (0,0,0)
        qk_chain(wk_sb, kTh, 0, 0, eager=True)
        qk_chain(wq_sb, qTh, 0, 0, eager=True)
        qk_chain(wq_sb, qTh, 0, 1, eager=True)
        emit_v(0, eager=True)
        emit_v(1, eager=True)

        # bg, ordered by first-use time (block0 sweeps k-ics / V jts as it
        # walks jt; pair0 q(c2=1) needed at block 2 = iter 16; pair1 at
        # block 4 = iter 32)
        qk_chain(wk_sb, kTh, 0, 1, eager=False)
        emit_v(2, eager=False)
        emit_v(3, eager=False)
        qk_chain(wk_sb, kTh, 0, 2, eager=False)
        emit_v(4, eager=False)
        qk_chain(wk_sb, kTh, 0, 3, eager=False)
        for jt in range(5, JT):
            emit_v(jt, eager=False)
        qk_chain(wq_sb, qTh, 0, 2, eager=False)
        qk_chain(wq_sb, qTh, 0, 3, eager=False)
        for ic in range(NIC):
            qk_chain(wk_sb, kTh, 1, ic, eager=False)
        for ic in range(NIC):
            qk_chain(wq_sb, qTh, 1, ic, eager=False)

        # blocks: pair0 both c2 first (pair1 projections then aren't needed
        # until iter 32); wo half0 ready after block 5, drains during blocks
        # 6-7; only wo half1 is epilogue.
        order = [(0, 0, 0), (0, 1, 0), (0, 0, 1), (0, 1, 1),
                 (1, 0, 0), (1, 1, 0), (1, 0, 1), (1, 1, 1)]
        for bi, (p, hh, c2) in enumerate(order):
            drain = 12 if bi < 2 else (8 if bi < 4 else 4)
            attn_block(p, hh, c2, drain)
            if bi == 1:
                pop_pend(len(pend))  # block1 norm before wo_A reads attnT[0]
                queue_wo_A(0)
            elif bi == 3:
                pop_pend(len(pend))
                queue_wo_A(1)
            elif bi == 5:
                pop_pend(len(pend))  # block5 norm before wo_B reads attnT[1]
                queue_wo_B(0)
        pop_pend(len(pend))
        drain_bg(len(bg))
        queue_wo_B(1)
        drain_bg(len(bg))

        import os
        if os.environ.get("ATTN_DEBUG"):
            at0_e = nc.dram_tensor("dbg_at0", [P, N], BF16, kind="ExternalOutput")
            at1_e = nc.dram_tensor("dbg_at1", [P, N], BF16, kind="ExternalOutput")
            nc.sync.dma_start(out=at0_e[:], in_=attnT[0][:])
            nc.sync.dma_start(out=at1_e[:], in_=attnT[1][:])


def build():
    if "nc" not in _compiled:
        nc = bacc.Bacc("TRN2", target_bir_lowering=False, debug=False,
                       num_devices=NCORES)
        with tile.TileContext(nc) as tc:
            _emit(tc)
        nc.compile()
        _compiled["nc"] = nc
    return _compiled["nc"]


def kernel(tokens, context_mask, Wq, Wkv, Wo, _profile=False):
    tokens = np.asarray(tokens, dtype=np.float32)
    Wq = np.asarray(Wq, dtype=np.float32)
    Wkv = np.asarray(Wkv, dtype=np.float32)
    Wo = np.asarray(Wo, dtype=np.float32)

    nc = build()
    scale = np.float32(HD ** -0.5)
    tokT = [np.ascontiguousarray(tokens[b].T).astype(NPBF16) for b in range(B)]
    in_maps = []
    for core in range(NCORES):
        b, g = divmod(core, G)
        in_maps.append({
            "tokT": tokT[b],
            "wq": np.ascontiguousarray(
                (Wq[:, C * g:C * (g + 1)] * scale).astype(NPBF16)),
            "wk": np.ascontiguousarray(
                Wkv[:, C * g:C * (g + 1)].astype(NPBF16)),
            "wv": np.ascontiguousarray(
                Wkv[:, D + C * g:D + C * (g + 1)].astype(NPBF16)),
            "wo": np.ascontiguousarray(
                Wo[C * g:C * (g + 1), :].astype(NPBF16)),
        })
    kwargs = {}
    if _profile:
        kwargs = dict(trace=True,
                      tmpdir=_profile if isinstance(_profile, str) else None)
    res = run_bass_kernel_spmd(nc, in_maps, core_ids=list(range(NCORES)), **kwargs)

    out = np.zeros((B, N, D), dtype=np.float32)
    for core in range(NCORES):
        b = core // G
        out[b] += res.results[core]["out"].astype(np.float32)
    if _profile:
        return out, res
    return out
